# revision 1
# baseline (speedup 1.0000x reference)
"""AdaProp GNN message-passing kernel for 8 TRN2 NeuronCores.

Strategy: shard edges by destination-node range (6250 nodes per core) so the
segment-sum is fully local per core (no all-reduce). Precompute
  hG   = [hidden @ Ws | hidden @ Wh]        [N, 256]  (sharded build + AllGather)
  hrG  = [rela  @ Wr | rela  @ Wh]          [401, 256]
  hqr  = rela[q_rel] @ Wqr_w + Wqr_b        [64, 128]
Then per edge only three row gathers are needed (dma_gather, int16 indices —
hence the hG table is gathered as two <32768-row halves); the attention logit
is a fused relu-mul-accumulate on DVE; the segment sum is a one-hot
(alpha-scaled) matmul accumulating in PSUM; Wh is folded into the tables; the
final relu rides the DVE PSUM eviction.

Hardware constraint baked throughout: this walrus build allows at most ONE
semaphore wait per PE instruction, so every tile read by the TensorEngine is
last-written by the DVE and PSUM slots are recycled by DVE readers.
"""

import numpy as np

N, E, B, D = 50000, 500_000, 64, 128
NCORES = 8
NPC = 6250            # output nodes per core
WIN = 128             # nodes per PSUM window
NWIN = (NPC + WIN - 1) // WIN          # 49 windows per core
TBL_ROWS = NWIN * WIN                  # 6272 rows per hG slice
TBL_FULL = TBL_ROWS * NCORES           # 50176
HALF = TBL_FULL // 2                   # 25088 (< 32768 so int16 indices work)
G = 3                 # windows per gather group
P = 128


def _host_shard(edges):
    sub = np.asarray(edges[:, 4], dtype=np.int64)
    rel = np.asarray(edges[:, 2], dtype=np.int64)
    obj = np.asarray(edges[:, 5], dtype=np.int64)
    ridx = np.asarray(edges[:, 0], dtype=np.int64)

    core = obj // NPC
    loc = obj - core * NPC
    win = loc // WIN
    sel = loc - win * WIN
    half = (sub >= HALF).astype(np.int64)

    # per (core, window, half) edge index lists
    lists = [[[None, None] for _ in range(NWIN)] for _ in range(NCORES)]
    for k in range(NCORES):
        mk = np.nonzero(core == k)[0]
        key = win[mk] * 2 + half[mk]
        order = np.argsort(key, kind="stable")
        mk = mk[order]
        key = key[order]
        bounds = np.searchsorted(key, np.arange(2 * NWIN + 1))
        for w in range(NWIN):
            lists[k][w][0] = mk[bounds[2 * w]:bounds[2 * w + 1]]
            lists[k][w][1] = mk[bounds[2 * w + 1]:bounds[2 * w + 2]]

    # global per-(window,half) tile counts -> identical SPMD graph on all cores
    tcA = [max(len(lists[k][w][0]) for k in range(NCORES)) for w in range(NWIN)]
    tcB = [max(len(lists[k][w][1]) for k in range(NCORES)) for w in range(NWIN)]
    tcA = [(n + P - 1) // P for n in tcA]
    tcB = [(n + P - 1) // P for n in tcB]
    for w in range(NWIN):
        if tcA[w] + tcB[w] == 0:
            tcA[w] = 1

    # groups of G windows; tile stream per group: [A tiles][B tiles]
    groups = []          # (c_start, tilesA, tilesB, windowsA list, windowsB list)
    tile_window = []
    c = 0
    for g0 in range(0, NWIN, G):
        ws = list(range(g0, min(g0 + G, NWIN)))
        tA = sum(tcA[w] for w in ws)
        tB = sum(tcB[w] for w in ws)
        for w in ws:
            tile_window += [w] * tcA[w]
        for w in ws:
            tile_window += [w] * tcB[w]
        groups.append((c, tA, tB))
        c += tA + tB
    ctot = c
    S = ctot * P // 16   # idx array columns

    subs16 = np.zeros((NCORES, 16, S), dtype=np.int16)
    rels16 = np.zeros((NCORES, 16, S), dtype=np.int16)
    objs = np.full((NCORES, P, ctot), -1.0, dtype=np.float32)

    for k in range(NCORES):
        gi = 0
        for g0 in range(0, NWIN, G):
            ws = list(range(g0, min(g0 + G, NWIN)))
            c_start, tA, tB = groups[gi]
            gi += 1
            s0 = c_start * P // 16        # idx column base of this group
            n_all = (tA + tB) * P

            # build the group's slot-ordered edge list (A runs then B runs)
            slot_sub = np.zeros(n_all, dtype=np.int64)
            slot_rel = np.zeros(n_all, dtype=np.int64)
            slot_obj = np.full(n_all, -1.0, dtype=np.float32)
            pos = 0
            for h, tc in ((0, tcA), (1, tcB)):
                for w in ws:
                    idx = lists[k][w][h]
                    n = len(idx)
                    nt = tc[w] * P
                    if n:
                        slot_sub[pos:pos + n] = sub[idx]
                        slot_rel[pos:pos + n] = rel[idx] * 64 + ridx[idx]
                        slot_obj[pos:pos + n] = sel[idx]
                    # pad slots: harmless gather target in the right half
                    slot_sub[pos + n:pos + nt] = 0 if h == 0 else HALF
                    pos += nt

            # per-slot arrays in [p, c] layout (slot j -> p=j%128, c=j//128)
            j = np.arange(n_all)
            objs[k, j % P, c_start + j // P] = slot_obj
            # idx arrays in 16-partition wrap per gather run
            nA = tA * P
            jA = np.arange(nA)
            subs16[k, jA % 16, s0 + jA // 16] = slot_sub[:nA]
            jB = np.arange(n_all - nA)
            subs16[k, jB % 16, s0 + nA // 16 + jB // 16] = slot_sub[nA:] - HALF
            rels16[k, j % 16, s0 + j // 16] = slot_rel

    return subs16, rels16, objs, tile_window, groups, ctot


def _build_graph(ctot, tile_window, groups):
    import concourse.bass as bass
    import concourse.bacc as bacc
    import concourse.mybir as mybir
    from concourse.tile import TileContext
    from concourse.masks import make_identity

    f32 = mybir.dt.float32
    bf16 = mybir.dt.bfloat16
    i16 = mybir.dt.int16
    AF = mybir.ActivationFunctionType
    Alu = mybir.AluOpType

    S = ctot * P // 16

    nc = bacc.Bacc()
    hid_s = nc.declare_dram_parameter("hid_s", [TBL_ROWS, D], bf16, isOutput=False)
    rela = nc.declare_dram_parameter("rela", [401, D], f32, isOutput=False)
    qrel = nc.declare_dram_parameter("qrel", [64, D], f32, isOutput=False)
    ws = nc.declare_dram_parameter("ws", [D, D], f32, isOutput=False)
    wr = nc.declare_dram_parameter("wr", [D, D], f32, isOutput=False)
    wh = nc.declare_dram_parameter("wh", [D, D], f32, isOutput=False)
    wqr = nc.declare_dram_parameter("wqr", [D, D], f32, isOutput=False)
    wqrb = nc.declare_dram_parameter("wqrb", [1, D], f32, isOutput=False)
    wa = nc.declare_dram_parameter("wa", [1, D], f32, isOutput=False)
    sub_i = nc.declare_dram_parameter("sub_i", [16, S], i16, isOutput=False)
    rel_i = nc.declare_dram_parameter("rel_i", [16, S], i16, isOutput=False)
    obj_f = nc.declare_dram_parameter("obj_f", [P, ctot], bf16, isOutput=False)
    out_ext = nc.declare_dram_parameter("out", [TBL_ROWS, D], f32, isOutput=True)

    first_tile = {}
    last_tile = {}
    for c, w in enumerate(tile_window):
        if w not in first_tile:
            first_tile[w] = c
        last_tile[w] = c

    with TileContext(nc) as tc:
        with (
            tc.tile_pool(name="const", bufs=1) as cpool,
            tc.tile_pool(name="dram", bufs=1, space="DRAM") as dpool,
            tc.tile_pool(name="work", bufs=2) as wpool,
            tc.tile_pool(name="psum", bufs=2, space="PSUM") as ppool,
            tc.tile_pool(name="aggp", bufs=4, space="PSUM") as apool,
        ):
            # ---- constants ----
            ident_g = cpool.tile([P, P], f32)
            make_identity(nc, ident_g[:])
            ident = cpool.tile([P, P], f32)
            nc.vector.tensor_copy(ident[:], ident_g[:])
            iota_i = cpool.tile([P, P], mybir.dt.int32)
            nc.gpsimd.iota(iota_i[:], pattern=[[1, P]], base=0, channel_multiplier=0)
            iota_b = cpool.tile([P, P], bf16)
            nc.vector.tensor_copy(iota_b[:], iota_i[:])
            ones_g = cpool.tile([1, P], f32)
            nc.gpsimd.memset(ones_g[:], 1.0)
            ones_r = cpool.tile([1, P], bf16)
            nc.vector.tensor_copy(ones_r[:], ones_g[:])

            def load_bf16(dst, src_ap, tagname):
                t = wpool.tile(list(src_ap.shape), f32, tag="wload", name=f"wl_{tagname}")
                nc.sync.dma_start(out=t[:], in_=src_ap)
                nc.vector.tensor_copy(dst, t[:])

            wcat_g = cpool.tile([P, 2 * D], bf16)
            load_bf16(wcat_g[:, 0:D], ws[:], "ws")
            load_bf16(wcat_g[:, D:2 * D], wh[:], "wh1")
            wcat_r = cpool.tile([P, 2 * D], bf16)
            load_bf16(wcat_r[:, 0:D], wr[:], "wr")
            load_bf16(wcat_r[:, D:2 * D], wh[:], "wh2")
            wqr_b = cpool.tile([P, D], bf16)
            load_bf16(wqr_b[:], wqr[:], "wqr")
            bias_r = cpool.tile([1, D], bf16)
            load_bf16(bias_r[:], wqrb[:], "wqrb")
            wa_r = cpool.tile([1, D], bf16)
            load_bf16(wa_r[:], wa[:], "wa")

            wab_ps = ppool.tile([P, D], f32, tag="mm")
            nc.tensor.matmul(wab_ps[:], lhsT=ones_r[:], rhs=wa_r[:], start=True, stop=True)
            wab = cpool.tile([P, D], bf16)
            nc.vector.tensor_copy(wab[:], wab_ps[:])

            # ---- DRAM tables ----
            hG_slice = dpool.tile([TBL_ROWS, 2 * D], bf16)
            hG_full = dpool.tile([TBL_FULL, 2 * D], bf16, addr_space="Shared")
            hrG = dpool.tile([512, 2 * D], bf16)
            hqr_d = dpool.tile([P, D], bf16)

            # hG slice build: 49 tiles of [128, 256]
            identb = cpool.tile([P, P], bf16)
            nc.vector.tensor_copy(identb[:], ident_g[:])
            for i in range(NWIN):
                h_t = wpool.tile([P, D], bf16, tag="h_in_b")
                nc.sync.dma_start(out=h_t[:], in_=hid_s[i * P:(i + 1) * P, :])
                tr_ps = ppool.tile([P, P], bf16, tag="tr")
                nc.tensor.transpose(tr_ps[:], h_t[:], identb[:])
                hT = wpool.tile([P, P], bf16, tag="hT")
                nc.scalar.copy(hT[:], tr_ps[:])
                g_ps = ppool.tile([P, 2 * D], f32, tag="mm")
                nc.tensor.matmul(g_ps[:], lhsT=hT[:], rhs=wcat_g[:], start=True, stop=True)
                g_b = wpool.tile([P, 2 * D], bf16, tag="g_out")
                nc.scalar.copy(g_b[:], g_ps[:])
                nc.sync.dma_start(out=hG_slice[i * P:(i + 1) * P, :], in_=g_b[:])

            nc.gpsimd.collective_compute(
                "AllGather",
                mybir.AluOpType.bypass,
                replica_groups=[list(range(NCORES))],
                ins=[hG_slice[:]],
                outs=[hG_full[:]],
            )

            # hrG build: 4 tiles (401 rows padded to 512)
            for i in range(4):
                r_t = wpool.tile([P, D], f32, tag="h_in")
                lo = i * P
                hi = min(401, lo + P)
                if hi - lo < P:
                    nc.gpsimd.memset(r_t[:], 0.0)
                nc.sync.dma_start(out=r_t[0:hi - lo, :], in_=rela[lo:hi, :])
                tr_ps = ppool.tile([P, P], f32, tag="tr")
                nc.tensor.transpose(tr_ps[:], r_t[:], ident[:])
                rT = wpool.tile([P, P], bf16, tag="hT")
                nc.scalar.copy(rT[:], tr_ps[:])
                g_ps = ppool.tile([P, 2 * D], f32, tag="mm")
                nc.tensor.matmul(g_ps[:], lhsT=rT[:], rhs=wcat_r[:], start=True, stop=True)
                g_b = wpool.tile([P, 2 * D], bf16, tag="g_out")
                nc.scalar.copy(g_b[:], g_ps[:])
                nc.sync.dma_start(out=hrG[i * P:(i + 1) * P, :], in_=g_b[:])

            # hqr build
            q_t = wpool.tile([P, D], f32, tag="h_in")
            nc.gpsimd.memset(q_t[:], 0.0)
            nc.sync.dma_start(out=q_t[0:64, :], in_=qrel[:])
            tr_ps = ppool.tile([P, P], f32, tag="tr")
            nc.tensor.transpose(tr_ps[:], q_t[:], ident[:])
            qT = wpool.tile([P, P], bf16, tag="hT")
            nc.scalar.copy(qT[:], tr_ps[:])
            q_ps = ppool.tile([P, D], f32, tag="mm")
            nc.tensor.matmul(q_ps[:], lhsT=qT[:], rhs=wqr_b[:], start=True, stop=False)
            nc.tensor.matmul(q_ps[:], lhsT=ones_r[:], rhs=bias_r[:], start=False, stop=True)
            q_b = wpool.tile([P, D], bf16, tag="g_out")
            nc.scalar.copy(q_b[:], q_ps[:])
            nc.sync.dma_start(out=hqr_d[:], in_=q_b[:])

            # crel fused table: row c = rel*64 + ridx ->
            #   [ hrW[rel] + hqr[ridx] (+bias, folded in hqr) | hrWh[rel] ]
            crel_d = dpool.tile([401 * 64, 2 * D], bf16)
            for half0 in range(0, 401, 16):
                nr = min(401, half0 + 16) - half0
                src_rep = bass.AP(hrG[:].tensor, half0 * 2 * D,
                                  [[2 * D, nr], [0, 64], [1, 2 * D]])
                dst_rep = bass.AP(crel_d[:].tensor, half0 * 64 * 2 * D,
                                  [[64 * 2 * D, nr], [2 * D, 64], [1, 2 * D]])
                nc.sync.dma_start(out=dst_rep, in_=src_rep)
                src_q = bass.AP(hqr_d[:].tensor, 0, [[0, nr], [D, 64], [1, D]])
                dst_q = bass.AP(crel_d[:].tensor, half0 * 64 * 2 * D,
                                [[64 * 2 * D, nr], [2 * D, 64], [1, D]])
                nc.gpsimd.dma_start(out=dst_q, in_=src_q, accum_op=Alu.add)

            # ---- edge index arrays resident in SBUF ----
            sub_s = cpool.tile([P, S], i16)
            nc.sync.dma_start(out=sub_s[0:16, :], in_=sub_i[:])
            rel_s = cpool.tile([P, S], i16)
            nc.sync.dma_start(out=rel_s[0:16, :], in_=rel_i[:])
            for rr in range(1, 8):
                nc.sync.dma_start(out=sub_s[16 * rr:16 * (rr + 1), :], in_=sub_s[0:16, :])
                nc.sync.dma_start(out=rel_s[16 * rr:16 * (rr + 1), :], in_=rel_s[0:16, :])
            obj_s = cpool.tile([P, ctot], bf16)
            nc.sync.dma_start(out=obj_s[:], in_=obj_f[:])

            # ---- edge processing ----
            agg = {}
            for c_start, tA, tB in groups:
                T = tA + tB
                n_all = T * P
                nA = tA * P
                nB = tB * P
                s0 = c_start * P // 16

                MAXI = 1024   # dma_gather ucode limit on num_idxs per call

                def chunked_gather(dst_tile, src_ap, idxs_tile, idx_col0, t_off,
                                   n, elem):
                    done = 0
                    while done < n:
                        cn = min(MAXI, n - done)
                        ct0 = t_off + done // P
                        nc.gpsimd.dma_gather(
                            out_ap=dst_tile[:, ct0:ct0 + cn // P, :],
                            in_ap=src_ap,
                            idxs_ap=idxs_tile[:, idx_col0 + done // 16:
                                              idx_col0 + (done + cn) // 16],
                            num_idxs=cn, num_idxs_reg=cn, elem_size=elem)
                        done += cn

                g_t = wpool.tile([P, T, 2 * D], bf16, tag="g_g", bufs=3)
                if tA:
                    chunked_gather(g_t, hG_full[0:HALF, :], sub_s, s0, 0, nA, 2 * D)
                if tB:
                    chunked_gather(g_t, hG_full[HALF:TBL_FULL, :], sub_s,
                                   s0 + nA // 16, tA, nB, 2 * D)
                r_t = wpool.tile([P, T, 2 * D], bf16, tag="g_r", bufs=3)
                chunked_gather(r_t, crel_d[:], rel_s, s0, 0, n_all, 2 * D)

                x1 = wpool.tile([P, T, D], bf16, tag="x1")
                nc.vector.tensor_tensor(
                    out=x1[:], in0=g_t[:, :, 0:D], in1=r_t[:, :, 0:D], op=Alu.add)

                logit = wpool.tile([P, T], f32, tag="logit")
                dump = wpool.tile([P, D], bf16, tag="dump")
                for c in range(T):
                    nc.vector.scalar_tensor_tensor(
                        out=dump[:], in0=x1[:, c, :], scalar=0.0, in1=wab[:],
                        op0=Alu.max, op1=Alu.mult,
                        accum_out=logit[:, c:c + 1])
                alpha = wpool.tile([P, T], bf16, tag="alpha")
                nc.scalar.activation(alpha[:], logit[:], AF.Sigmoid)

                oh = wpool.tile([P, T, P], bf16, tag="oh")
                for c in range(T):
                    nc.vector.scalar_tensor_tensor(
                        out=oh[:, c, :], in0=iota_b[:],
                        scalar=obj_s[:, c_start + c:c_start + c + 1],
                        in1=alpha[:, c:c + 1].to_broadcast([P, P]),
                        op0=Alu.is_equal, op1=Alu.mult)

                for c in range(T):
                    ct = c_start + c
                    w = tile_window[ct]
                    if ct == first_tile[w]:
                        agg[w] = apool.tile([P, D], f32, tag="agg", name=f"agg_{w}")
                    nc.tensor.matmul(agg[w][:], lhsT=oh[:, c, :],
                                     rhs=g_t[:, c, D:2 * D],
                                     start=(ct == first_tile[w]), stop=False)
                    nc.tensor.matmul(agg[w][:], lhsT=oh[:, c, :],
                                     rhs=r_t[:, c, D:2 * D],
                                     start=False, stop=(ct == last_tile[w]))
                    if ct == last_tile[w]:
                        o_t = wpool.tile([P, D], f32, tag="o_t")
                        nc.scalar.activation(o_t[:], agg[w][:], AF.Relu)
                        nc.sync.dma_start(out=out_ext[w * P:(w + 1) * P, :], in_=o_t[:])
                        del agg[w]

    nc.compile()
    return nc


def kernel(q_rel, hidden, edges, rela_embed, Ws, Wr, Wqr_w, Wqr_b, Wa, Wh, n_node):
    from concourse.bass_utils import run_bass_kernel_spmd

    q_rel = np.asarray(q_rel)
    hidden = np.asarray(hidden, dtype=np.float32)
    edges = np.asarray(edges)
    rela_embed = np.asarray(rela_embed, dtype=np.float32)

    subs16, rels16, objs, tile_window, groups, ctot = _host_shard(edges)
    nc = _build_graph(ctot, tile_window, groups)

    import ml_dtypes
    hid_pad = np.zeros((TBL_FULL, D), dtype=ml_dtypes.bfloat16)
    hid_pad[:N] = hidden.astype(ml_dtypes.bfloat16)
    qrel_sel = np.ascontiguousarray(rela_embed[np.asarray(q_rel, dtype=np.int64)])

    in_maps = []
    for k in range(NCORES):
        in_maps.append({
            "hid_s": np.ascontiguousarray(hid_pad[k * TBL_ROWS:(k + 1) * TBL_ROWS]),
            "rela": rela_embed,
            "qrel": qrel_sel,
            "ws": np.asarray(Ws, dtype=np.float32),
            "wr": np.asarray(Wr, dtype=np.float32),
            "wh": np.asarray(Wh, dtype=np.float32),
            "wqr": np.asarray(Wqr_w, dtype=np.float32),
            "wqrb": np.asarray(Wqr_b, dtype=np.float32).reshape(1, D),
            "wa": np.asarray(Wa, dtype=np.float32).reshape(1, D),
            "sub_i": subs16[k],
            "rel_i": rels16[k],
            "obj_f": objs[k].astype(__import__("ml_dtypes").bfloat16),
        })

    res = run_bass_kernel_spmd(nc, in_maps, list(range(NCORES)))
    out = np.concatenate([res.results[k]["out"][:NPC] for k in range(NCORES)], axis=0)
    return out.astype(np.float32)


if __name__ == "__main__":
    import reference

    inputs = reference.setup_inputs()
    inputs = {k: np.asarray(v) for k, v in inputs.items()}
    got = kernel(**inputs)
    exp = np.asarray(reference.reference(**reference.setup_inputs()))
    err = np.abs(got - exp).max() / (np.abs(exp).max() + 1e-9)
    print("rel err:", err)



# revision 18
# speedup vs baseline: 1.3212x; 1.3212x over previous
"""AdaProp GNN message-passing kernel for 8 TRN2 NeuronCores.

Strategy (v2 — collective-free): edges are sharded by destination-node range
(6250 nodes per core) so the segment-sum is fully local. Every core receives
the FULL transposed hidden state and builds the full projection table
  hG   = [hidden @ Ws' | hidden @ Wh]   [50176, 256] bf16  (A/B split halves)
locally on the TensorEngine (no AllGather). |Wa| is folded into Ws/Wr/Wqr/b
columns, which are permuted so positive-sign Wa columns come first; the
attention logit is then two 4x-mode tensor_scalar relu-accumulates
(l1 - l2 = Wa . relu(pre)). The relation table
  crel = [rela@Wr' + hqr' (by rel*64+ridx) | rela@Wh]   [25728, 256] bf16
is built by PE matmuls against constant selector matrices. Per edge, two
512-byte dma_gather rows (hG by sub, crel by rel*64+r_idx) are fetched and
summed in place on DVE; the alpha-scaled one-hot of the destination node is a
single two-scalar tensor_scalar (is_equal, mult); the segment sum is one
PSUM-accumulated matmul per tile; relu rides the Activation-engine eviction.
"""

import numpy as np

N, E, B, D = 50000, 500_000, 64, 128
NCORES = 8
NPC = 6250              # output nodes per core
WIN = 128               # nodes per PSUM window
NWIN = (NPC + WIN - 1) // WIN           # 49 windows per core
OUT_ROWS = NWIN * WIN                   # 6272 output rows per core
ROWS_T = 50176                          # hG table rows (50000 padded)
HALF = ROWS_T // 2                      # 25088 (< 32768 so int16 idx works)
NT_H = HALF // 128                      # 196 tiles per half table
BCH = 14                                # hG build tiles per DMA batch (196=14*14)
CREL_T = 201                            # crel tiles (201*128 = 25728 >= 401*64)
CREL_ROWS = CREL_T * 128
G = 3                                   # windows per gather group
MAXI = 1024                             # max idxs per dma_gather call
P = 128


def _host_shard(edges):
    sub = np.asarray(edges[:, 4], dtype=np.int64)
    rel = np.asarray(edges[:, 2], dtype=np.int64)
    obj = np.asarray(edges[:, 5], dtype=np.int64)
    ridx = np.asarray(edges[:, 0], dtype=np.int64)

    core = obj // NPC
    loc = obj - core * NPC
    win = loc // WIN
    sel = loc - win * WIN
    half = (sub >= HALF).astype(np.int64)

    # per (core, window, half) edge index lists
    lists = [[[None, None] for _ in range(NWIN)] for _ in range(NCORES)]
    for k in range(NCORES):
        mk = np.nonzero(core == k)[0]
        key = win[mk] * 2 + half[mk]
        order = np.argsort(key, kind="stable")
        mk = mk[order]
        key = key[order]
        bounds = np.searchsorted(key, np.arange(2 * NWIN + 1))
        for w in range(NWIN):
            lists[k][w][0] = mk[bounds[2 * w]:bounds[2 * w + 1]]
            lists[k][w][1] = mk[bounds[2 * w + 1]:bounds[2 * w + 2]]

    # global per-(window,half) tile counts -> identical SPMD graph on all cores
    tcA = [max(len(lists[k][w][0]) for k in range(NCORES)) for w in range(NWIN)]
    tcB = [max(len(lists[k][w][1]) for k in range(NCORES)) for w in range(NWIN)]
    tcA = [(n + P - 1) // P for n in tcA]
    tcB = [(n + P - 1) // P for n in tcB]
    for w in range(NWIN):
        if tcA[w] + tcB[w] == 0:
            tcA[w] = 1

    # groups of G windows; tile stream per group: [A tiles][B tiles]
    groups = []          # (c_start, tA, tB)
    tile_window = []
    c = 0
    for g0 in range(0, NWIN, G):
        ws = list(range(g0, min(g0 + G, NWIN)))
        tA = sum(tcA[w] for w in ws)
        tB = sum(tcB[w] for w in ws)
        for w in ws:
            tile_window += [w] * tcA[w]
        for w in ws:
            tile_window += [w] * tcB[w]
        groups.append((c, tA, tB))
        c += tA + tB
    ctot = c
    S = ctot * P // 16   # idx array columns

    subs16 = np.zeros((NCORES, 16, S), dtype=np.int16)
    rels16 = np.zeros((NCORES, 16, S), dtype=np.int16)
    objs = np.full((NCORES, P, ctot), -1.0, dtype=np.float32)

    for k in range(NCORES):
        gi = 0
        for g0 in range(0, NWIN, G):
            ws = list(range(g0, min(g0 + G, NWIN)))
            c_start, tA, tB = groups[gi]
            gi += 1
            s0 = c_start * P // 16        # idx column base of this group
            n_all = (tA + tB) * P
            nA = tA * P

            # build the group's slot-ordered edge list (A runs then B runs)
            slot_sub = np.zeros(n_all, dtype=np.int64)
            slot_rel = np.zeros(n_all, dtype=np.int64)
            slot_obj = np.full(n_all, -1.0, dtype=np.float32)
            pos = 0
            for h, tc in ((0, tcA), (1, tcB)):
                for w in ws:
                    idx = lists[k][w][h]
                    n = len(idx)
                    nt = tc[w] * P
                    if n:
                        slot_sub[pos:pos + n] = sub[idx]
                        slot_rel[pos:pos + n] = rel[idx] * 64 + ridx[idx]
                        slot_obj[pos:pos + n] = sel[idx]
                    # pad slots: harmless gather target in the right half
                    slot_sub[pos + n:pos + nt] = 0 if h == 0 else HALF
                    pos += nt

            # per-slot arrays in [p, c] layout (slot j -> p=j%128, c=j//128)
            j = np.arange(n_all)
            objs[k, j % P, c_start + j // P] = slot_obj
            # idx arrays in 16-partition wrap, one wrap run per half segment
            jA = np.arange(nA)
            jB = np.arange(n_all - nA)
            subs16[k, jA % 16, s0 + jA // 16] = slot_sub[:nA]
            subs16[k, jB % 16, s0 + nA // 16 + jB // 16] = slot_sub[nA:] - HALF
            rels16[k, jA % 16, s0 + jA // 16] = slot_rel[:nA]
            rels16[k, jB % 16, s0 + nA // 16 + jB // 16] = slot_rel[nA:]

    subs16 = np.tile(subs16, (1, 8, 1))   # replicate to 128 partitions
    rels16 = np.tile(rels16, (1, 8, 1))
    return subs16, rels16, objs, tile_window, groups, ctot


DEBUG_OUTPUTS = False


def _build_graph(ctot, tile_window, groups, kpos):
    import concourse.bass as bass
    import concourse.bacc as bacc
    import concourse.mybir as mybir
    from concourse.tile import TileContext

    f32 = mybir.dt.float32
    bf16 = mybir.dt.bfloat16
    i16 = mybir.dt.int16
    AF = mybir.ActivationFunctionType
    Alu = mybir.AluOpType

    S = ctot * P // 16
    assert 2 <= kpos <= 126

    nc = bacc.Bacc(dynamic_dma_scratch_size=65536)
    hidT = nc.declare_dram_parameter("hidT", [P, ROWS_T], bf16, isOutput=False)
    relaT = nc.declare_dram_parameter("relaT", [P, 512], bf16, isOutput=False)
    qrelT = nc.declare_dram_parameter("qrelT", [P, 64], bf16, isOutput=False)
    ws_p = nc.declare_dram_parameter("ws_p", [D, D], bf16, isOutput=False)
    wr_p = nc.declare_dram_parameter("wr_p", [D, D], bf16, isOutput=False)
    wh_p = nc.declare_dram_parameter("wh_p", [D, D], bf16, isOutput=False)
    wqr_p = nc.declare_dram_parameter("wqr_p", [D, D], bf16, isOutput=False)
    b_p = nc.declare_dram_parameter("b_p", [1, D], bf16, isOutput=False)
    sub_i = nc.declare_dram_parameter("sub_i", [P, S], i16, isOutput=False)
    rel_i = nc.declare_dram_parameter("rel_i", [P, S], i16, isOutput=False)
    obj_f = nc.declare_dram_parameter("obj_f", [P, ctot], f32, isOutput=False)
    out_ext = nc.declare_dram_parameter("out", [OUT_ROWS, D], f32, isOutput=True)

    first_tile = {}
    last_tile = {}
    for c, w in enumerate(tile_window):
        if w not in first_tile:
            first_tile[w] = c
        last_tile[w] = c

    with TileContext(nc) as tc:
        with (
            tc.tile_pool(name="const", bufs=1) as cpool,
            tc.tile_pool(name="dram", bufs=1, space="DRAM") as dpool,
            tc.tile_pool(name="work", bufs=2) as wpool,
            tc.tile_pool(name="psum", bufs=4, space="PSUM") as ppool,
            tc.tile_pool(name="aggp", bufs=4, space="PSUM") as apool,
        ):
            # ---- constants ----
            chan_i = cpool.tile([P, 1], mybir.dt.int32)
            nc.gpsimd.iota(chan_i[:], pattern=[[0, 1]], base=0, channel_multiplier=1)
            chan_f = cpool.tile([P, 1], f32)
            nc.vector.tensor_copy(chan_f[:], chan_i[:])

            iota_i = cpool.tile([P, P], i16)
            nc.gpsimd.iota(iota_i[:], pattern=[[1, P]], base=0, channel_multiplier=0)
            iota_b = cpool.tile([P, P], bf16)
            nc.vector.tensor_copy(iota_b[:], iota_i[:])

            # E64[k, 64k'+b] = (k == k') (crel rel selector, sliced on free dim)
            e64 = cpool.tile([P, P * 64], bf16)
            e_scr = cpool.tile([P, 32 * 64], i16)
            e_scrb = cpool.tile([P, 32 * 64], bf16)
            for q in range(4):
                nc.gpsimd.iota(e_scr[:], pattern=[[1, 32], [0, 64]],
                               base=32 * q, channel_multiplier=0)
                nc.vector.tensor_copy(e_scrb[:], e_scr[:])
                nc.vector.tensor_scalar(
                    out=e64[:, q * 2048:(q + 1) * 2048], in0=e_scrb[:],
                    scalar1=chan_f[:], scalar2=None, op0=Alu.is_equal)

            # I64dup[q, j] = (q == j % 64)
            i64_i = cpool.tile([64, P], i16)
            nc.gpsimd.iota(i64_i[:], pattern=[[0, 2], [1, 64]], base=0,
                           channel_multiplier=0)
            i64_b = cpool.tile([64, P], bf16)
            nc.vector.tensor_copy(i64_b[:], i64_i[:])
            i64dup = cpool.tile([64, P], bf16)
            nc.vector.tensor_scalar(out=i64dup[:], in0=i64_b[:],
                                    scalar1=chan_f[0:64, :], scalar2=None,
                                    op0=Alu.is_equal)

            ones64 = cpool.tile([1, 64], bf16)
            nc.gpsimd.memset(ones64[:], 1.0)

            # ---- weights ----
            wcat_g = cpool.tile([P, 2 * D], bf16)
            nc.sync.dma_start(out=wcat_g[:, 0:D], in_=ws_p[:])
            nc.sync.dma_start(out=wcat_g[:, D:2 * D], in_=wh_p[:])
            wcat_r = cpool.tile([P, 2 * D], bf16)
            nc.sync.dma_start(out=wcat_r[:, 0:D], in_=wr_p[:])
            nc.sync.dma_start(out=wcat_r[:, D:2 * D], in_=wh_p[:])
            wqr_sb = cpool.tile([P, D], bf16)
            nc.sync.dma_start(out=wqr_sb[:], in_=wqr_p[:])
            bp_sb = cpool.tile([1, D], bf16)
            nc.sync.dma_start(out=bp_sb[:], in_=b_p[:])
            qrelT_sb = cpool.tile([P, 64], bf16)
            nc.sync.dma_start(out=qrelT_sb[:], in_=qrelT[:])
            relaT_sb = cpool.tile([P, 512], bf16)
            nc.sync.dma_start(out=relaT_sb[:], in_=relaT[:])

            # ---- edge index arrays ----
            sub_s = cpool.tile([P, S], i16)
            nc.sync.dma_start(out=sub_s[:], in_=sub_i[:])
            rel_s = cpool.tile([P, S], i16)
            nc.sync.dma_start(out=rel_s[:], in_=rel_i[:])
            obj_s = cpool.tile([P, ctot], f32)
            nc.sync.dma_start(out=obj_s[:], in_=obj_f[:])

            # ---- DRAM tables ----
            if DEBUG_OUTPUTS:
                hG_A = nc.declare_dram_parameter("dbg_hga", [HALF, 2 * D], bf16,
                                                 isOutput=True)
                hG_B = nc.declare_dram_parameter("dbg_hgb", [HALF, 2 * D], bf16,
                                                 isOutput=True)
                crel_d = nc.declare_dram_parameter("dbg_crel", [CREL_ROWS, 2 * D],
                                                   bf16, isOutput=True)
                dbg_l = nc.declare_dram_parameter("dbg_l", [P, 3 * ctot], f32,
                                                  isOutput=True)
            else:
                hG_A = dpool.tile([HALF, 2 * D], bf16)
                hG_B = dpool.tile([HALF, 2 * D], bf16)
                crel_d = dpool.tile([CREL_ROWS, 2 * D], bf16)

            # hrG in SBUF: [r_chunk, 256] x 4 chunks (512 rel rows padded)
            hrg_sb = cpool.tile([P, 4, 2 * D], bf16)
            for c in range(4):
                ps = ppool.tile([P, 2 * D], f32, tag="mm")
                nc.tensor.matmul(ps[:], lhsT=relaT_sb[:, c * P:(c + 1) * P],
                                 rhs=wcat_r[:], start=True, stop=True)
                nc.scalar.copy(hrg_sb[:, c, :], ps[:])

            # hqr' = qrel_sel @ Wqr' + b' -> [64, 256] (zero second half)
            hqr256 = cpool.tile([64, 2 * D], bf16)
            nc.gpsimd.memset(hqr256[:], 0.0)
            q_ps = ppool.tile([P, 2 * D], f32, tag="mm")
            nc.tensor.matmul(q_ps[0:64, 0:D], lhsT=qrelT_sb[:], rhs=wqr_sb[:],
                             start=True, stop=False)
            nc.tensor.matmul(q_ps[0:64, 0:D], lhsT=ones64[:], rhs=bp_sb[:],
                             start=False, stop=True)
            nc.scalar.copy(hqr256[:, 0:D], q_ps[0:64, 0:D])

            def batched_store(dst_tensor, row0, stage, nt):
                ap = bass.AP(dst_tensor, row0 * 2 * D,
                             [[2 * D, P], [P * 2 * D, nt], [1, 2 * D]])
                nc.sync.dma_start(out=ap, in_=stage[:, 0:nt, :])

            # ---- crel table build (PE) ----
            for t0 in range(0, CREL_T, 8):
                nt = min(8, CREL_T - t0)
                stage = wpool.tile([P, 8, 2 * D], bf16, tag="stage_c")
                for j in range(nt):
                    t = t0 + j
                    r0m = (2 * t) % P
                    chunk = (2 * t) // P
                    ps = ppool.tile([P, 2 * D], f32, tag="mm")
                    nc.tensor.matmul(ps[:], lhsT=e64[:, r0m * 64:r0m * 64 + P],
                                     rhs=hrg_sb[:, chunk, :],
                                     start=True, stop=False)
                    nc.tensor.matmul(ps[:], lhsT=i64dup[:], rhs=hqr256[:],
                                     start=False, stop=True)
                    nc.scalar.copy(stage[:, j, :], ps[:])
                batched_store(crel_d[:].tensor, t0 * P, stage, nt)

            # ---- hG table build (PE), half A then half B ----
            for half_i, hG in ((0, hG_A), (1, hG_B)):
                for b0 in range(0, NT_H, BCH):
                    h_t = wpool.tile([P, BCH * P], bf16, tag="h_in")
                    col0 = (half_i * NT_H + b0) * P
                    nc.sync.dma_start(out=h_t[:], in_=hidT[:, col0:col0 + BCH * P])
                    stage = wpool.tile([P, BCH, 2 * D], bf16, tag="stage_g")
                    for j in range(BCH):
                        ps = ppool.tile([P, 2 * D], f32, tag="mm")
                        nc.tensor.matmul(ps[:], lhsT=h_t[:, j * P:(j + 1) * P],
                                         rhs=wcat_g[:], start=True, stop=True)
                        nc.scalar.copy(stage[:, j, :], ps[:])
                    batched_store(hG[:].tensor, b0 * P, stage, BCH)

            # ---- per-edge-slot accumulators ----
            l1 = cpool.tile([P, ctot], f32)
            l2 = cpool.tile([P, ctot], f32)
            lg = cpool.tile([P, ctot], f32)
            alpha = cpool.tile([P, ctot], f32)

            def chunked_gather(dst_tile, src_ap, idxs_tile, idx_col0, t_off, n):
                done = 0
                while done < n:
                    cn = min(MAXI, n - done)
                    ct0 = t_off + done // P
                    nc.gpsimd.dma_gather(
                        out_ap=dst_tile[:, ct0:ct0 + cn // P, :],
                        in_ap=src_ap,
                        idxs_ap=idxs_tile[:, idx_col0 + done // 16:
                                          idx_col0 + (done + cn) // 16],
                        num_idxs=cn, num_idxs_reg=cn, elem_size=2 * D)
                    done += cn

            # ---- edge processing ----
            agg = {}
            for g_idx, (c_start, tA, tB) in enumerate(groups):
                g0w = g_idx * G
                T = tA + tB
                nA = tA * P
                nB = tB * P
                s0 = c_start * P // 16

                r_t = wpool.tile([P, T, 2 * D], bf16, tag="g_r")
                # s = g + r in place, one half-table segment at a time
                if tA:
                    gA = wpool.tile([P, tA, 2 * D], bf16, tag="g_g")
                    chunked_gather(gA, hG_A[:], sub_s, s0, 0, nA)
                    chunked_gather(r_t, crel_d[:], rel_s, s0, 0, nA)
                    nc.vector.tensor_tensor(out=r_t[:, 0:tA, :], in0=r_t[:, 0:tA, :],
                                            in1=gA[:, 0:tA, :], op=Alu.add)
                if tB:
                    gB = wpool.tile([P, tB, 2 * D], bf16, tag="g_g")
                    chunked_gather(gB, hG_B[:], sub_s, s0 + nA // 16, 0, nB)
                    chunked_gather(r_t, crel_d[:], rel_s, s0 + nA // 16, tA, nB)
                    nc.vector.tensor_tensor(out=r_t[:, tA:T, :], in0=r_t[:, tA:T, :],
                                            in1=gB[:, 0:tB, :], op=Alu.add)

                dump = wpool.tile([P, P], bf16, tag="dump")
                for c in range(T):
                    ct = c_start + c
                    nc.vector.tensor_scalar(
                        out=dump[:, 0:kpos], in0=r_t[:, c, 0:kpos],
                        scalar1=0.0, scalar2=0.0, op0=Alu.max, op1=Alu.add,
                        accum_out=l1[:, ct:ct + 1])
                    nc.vector.tensor_scalar(
                        out=dump[:, kpos:D], in0=r_t[:, c, kpos:D],
                        scalar1=0.0, scalar2=0.0, op0=Alu.max, op1=Alu.add,
                        accum_out=l2[:, ct:ct + 1])

                nc.vector.tensor_tensor(
                    out=lg[:, c_start:c_start + T], in0=l1[:, c_start:c_start + T],
                    in1=l2[:, c_start:c_start + T], op=Alu.subtract)
                nc.scalar.activation(alpha[:, c_start:c_start + T],
                                     lg[:, c_start:c_start + T], AF.Sigmoid)

                nw = len(set(tile_window[c_start:c_start + T]))
                ostage = wpool.tile([P, G, D], f32, tag="ostage")
                for c in range(T):
                    ct = c_start + c
                    oh = wpool.tile([P, P], bf16, tag="oh", bufs=4)
                    nc.vector.tensor_scalar(
                        out=oh[:], in0=iota_b[:],
                        scalar1=obj_s[:, ct:ct + 1], scalar2=alpha[:, ct:ct + 1],
                        op0=Alu.is_equal, op1=Alu.mult)
                    w = tile_window[ct]
                    if ct == first_tile[w]:
                        agg[w] = apool.tile([P, D], f32, tag="agg", name=f"agg_{w}")
                    nc.tensor.matmul(agg[w][:], lhsT=oh[:],
                                     rhs=r_t[:, c, D:2 * D],
                                     start=(ct == first_tile[w]),
                                     stop=(ct == last_tile[w]))
                    if ct == last_tile[w]:
                        nc.scalar.activation(ostage[:, w - g0w, :], agg[w][:],
                                             AF.Relu)
                        del agg[w]
                out_ap = bass.AP(out_ext[:].tensor, g0w * P * D,
                                 [[D, P], [P * D, nw], [1, D]])
                nc.sync.dma_start(out=out_ap, in_=ostage[:, 0:nw, :])

            if DEBUG_OUTPUTS:
                nc.sync.dma_start(out=dbg_l[:, 0:ctot], in_=l1[:])
                nc.sync.dma_start(out=dbg_l[:, ctot:2 * ctot], in_=l2[:])
                nc.sync.dma_start(out=dbg_l[:, 2 * ctot:3 * ctot], in_=alpha[:])

    nc.compile()
    return nc


def _prep_weights(Wa, Ws, Wr, Wqr_w, Wqr_b):
    import ml_dtypes
    wa = np.asarray(Wa, dtype=np.float64)
    pos = np.nonzero(wa >= 0)[0]
    neg = np.nonzero(wa < 0)[0]
    order = np.concatenate([pos, neg])
    kpos = len(pos)
    scale = np.abs(wa)[order]

    def prep(w):
        w = np.asarray(w, dtype=np.float64)[:, order] * scale[None, :]
        return np.ascontiguousarray(w.astype(ml_dtypes.bfloat16))

    ws_p = prep(Ws)
    wr_p = prep(Wr)
    wqr_p = prep(Wqr_w)
    b_p = (np.asarray(Wqr_b, dtype=np.float64)[order] * scale).reshape(1, D)
    b_p = np.ascontiguousarray(b_p.astype(ml_dtypes.bfloat16))
    return ws_p, wr_p, wqr_p, b_p, kpos


def prepare(q_rel, hidden, edges, rela_embed, Ws, Wr, Wqr_w, Wqr_b, Wa, Wh,
            n_node=None):
    """Build the Bass graph and the 8 per-core input maps."""
    import ml_dtypes

    q_rel = np.asarray(q_rel)
    hidden = np.asarray(hidden, dtype=np.float32)
    edges = np.asarray(edges)
    rela_embed = np.asarray(rela_embed, dtype=np.float32)

    subs16, rels16, objs, tile_window, groups, ctot = _host_shard(edges)
    ws_p, wr_p, wqr_p, b_p, kpos = _prep_weights(Wa, Ws, Wr, Wqr_w, Wqr_b)
    nc = _build_graph(ctot, tile_window, groups, kpos)

    bf = ml_dtypes.bfloat16
    hidT = np.zeros((D, ROWS_T), dtype=bf)
    hidT[:, :N] = hidden.T.astype(bf)
    relaT = np.zeros((D, 512), dtype=bf)
    relaT[:, :rela_embed.shape[0]] = rela_embed.T.astype(bf)
    qrelT = np.ascontiguousarray(
        rela_embed[np.asarray(q_rel, dtype=np.int64)].T.astype(bf))
    wh_b = np.ascontiguousarray(np.asarray(Wh, dtype=np.float32).astype(bf))

    in_maps = []
    for k in range(NCORES):
        in_maps.append({
            "hidT": hidT,
            "relaT": relaT,
            "qrelT": qrelT,
            "ws_p": ws_p,
            "wr_p": wr_p,
            "wh_p": wh_b,
            "wqr_p": wqr_p,
            "b_p": b_p,
            "sub_i": subs16[k],
            "rel_i": rels16[k],
            "obj_f": objs[k],
        })
    return nc, in_maps


def kernel(q_rel, hidden, edges, rela_embed, Ws, Wr, Wqr_w, Wqr_b, Wa, Wh, n_node):
    from concourse.bass_utils import run_bass_kernel_spmd

    nc, in_maps = prepare(q_rel, hidden, edges, rela_embed, Ws, Wr, Wqr_w,
                          Wqr_b, Wa, Wh, n_node)
    res = run_bass_kernel_spmd(nc, in_maps, list(range(NCORES)))
    out = np.concatenate([res.results[k]["out"][:NPC] for k in range(NCORES)],
                         axis=0)
    return out.astype(np.float32)


if __name__ == "__main__":
    import reference

    inputs = reference.setup_inputs()
    inputs = {k: np.asarray(v) for k, v in inputs.items()}
    got = kernel(**inputs)
    exp = np.asarray(reference.reference(**inputs))
    err = np.abs(got - exp).max() / (np.abs(exp).max() + 1e-9)
    print("rel err:", err)


# revision 23
# speedup vs baseline: 1.4629x; 1.1073x over previous
"""AdaProp GNN message-passing kernel for 8 TRN2 NeuronCores.

Strategy (v2 — collective-free): edges are sharded by destination-node range
(6250 nodes per core) so the segment-sum is fully local. Every core receives
the FULL transposed hidden state and builds the full projection table
  hG   = [hidden @ Ws' | hidden @ Wh]   [50176, 256] bf16  (A/B split halves)
locally on the TensorEngine (no AllGather). |Wa| is folded into Ws/Wr/Wqr/b
columns, which are permuted so positive-sign Wa columns come first; the
attention logit is then two 4x-mode tensor_scalar relu-accumulates
(l1 - l2 = Wa . relu(pre)). The relation table
  crel = [rela@Wr' + hqr' (by rel*64+ridx) | rela@Wh]   [25728, 256] bf16
is built by PE matmuls against constant selector matrices. Per edge, two
512-byte dma_gather rows (hG by sub, crel by rel*64+r_idx) are fetched and
summed in place on DVE; the alpha-scaled one-hot of the destination node is a
single two-scalar tensor_scalar (is_equal, mult); the segment sum is one
PSUM-accumulated matmul per tile; relu rides the Activation-engine eviction.
"""

import numpy as np

N, E, B, D = 50000, 500_000, 64, 128
NCORES = 8
NPC = 6250              # output nodes per core
WIN = 128               # nodes per PSUM window
NWIN = (NPC + WIN - 1) // WIN           # 49 windows per core
OUT_ROWS = NWIN * WIN                   # 6272 output rows per core
ROWS_T = 50176                          # hG table rows (50000 padded)
HALF = ROWS_T // 2                      # 25088 (< 32768 so int16 idx works)
NT_H = HALF // 128                      # 196 tiles per half table
BCH = 14                                # hG build tiles per DMA batch (196=14*14)
CREL_T = 201                            # crel tiles (201*128 = 25728 >= 401*64)
CREL_ROWS = CREL_T * 128
G = 3                                   # windows per gather group
MAXI = 1024                             # max idxs per dma_gather call (HW ucode limit)
P = 128


def _host_shard(edges):
    sub = np.asarray(edges[:, 4], dtype=np.int64)
    rel = np.asarray(edges[:, 2], dtype=np.int64)
    obj = np.asarray(edges[:, 5], dtype=np.int64)
    ridx = np.asarray(edges[:, 0], dtype=np.int64)

    core = obj // NPC
    loc = obj - core * NPC
    win = loc // WIN
    sel = loc - win * WIN
    half = (sub >= HALF).astype(np.int64)

    # per (core, window, half) edge index lists
    lists = [[[None, None] for _ in range(NWIN)] for _ in range(NCORES)]
    for k in range(NCORES):
        mk = np.nonzero(core == k)[0]
        key = win[mk] * 2 + half[mk]
        order = np.argsort(key, kind="stable")
        mk = mk[order]
        key = key[order]
        bounds = np.searchsorted(key, np.arange(2 * NWIN + 1))
        for w in range(NWIN):
            lists[k][w][0] = mk[bounds[2 * w]:bounds[2 * w + 1]]
            lists[k][w][1] = mk[bounds[2 * w + 1]:bounds[2 * w + 2]]

    # global per-(window,half) tile counts -> identical SPMD graph on all cores
    tcA = [max(len(lists[k][w][0]) for k in range(NCORES)) for w in range(NWIN)]
    tcB = [max(len(lists[k][w][1]) for k in range(NCORES)) for w in range(NWIN)]
    tcA = [(n + P - 1) // P for n in tcA]
    tcB = [(n + P - 1) // P for n in tcB]
    for w in range(NWIN):
        if tcA[w] + tcB[w] == 0:
            tcA[w] = 1

    # groups of G windows; tile stream per group: [A tiles][B tiles]
    groups = []          # (c_start, tA, tB)
    tile_window = []
    c = 0
    for g0 in range(0, NWIN, G):
        ws = list(range(g0, min(g0 + G, NWIN)))
        tA = sum(tcA[w] for w in ws)
        tB = sum(tcB[w] for w in ws)
        for w in ws:
            tile_window += [w] * tcA[w]
        for w in ws:
            tile_window += [w] * tcB[w]
        groups.append((c, tA, tB))
        c += tA + tB
    ctot = c
    S = ctot * P // 16   # idx array columns

    subs16 = np.zeros((NCORES, 16, S), dtype=np.int16)
    rels16 = np.zeros((NCORES, 16, S), dtype=np.int16)
    objs = np.full((NCORES, P, ctot), -1.0, dtype=np.float32)

    for k in range(NCORES):
        gi = 0
        for g0 in range(0, NWIN, G):
            ws = list(range(g0, min(g0 + G, NWIN)))
            c_start, tA, tB = groups[gi]
            gi += 1
            s0 = c_start * P // 16        # idx column base of this group
            n_all = (tA + tB) * P
            nA = tA * P

            # build the group's slot-ordered edge list (A runs then B runs)
            slot_sub = np.zeros(n_all, dtype=np.int64)
            slot_rel = np.zeros(n_all, dtype=np.int64)
            slot_obj = np.full(n_all, -1.0, dtype=np.float32)
            pos = 0
            for h, tc in ((0, tcA), (1, tcB)):
                for w in ws:
                    idx = lists[k][w][h]
                    n = len(idx)
                    nt = tc[w] * P
                    if n:
                        slot_sub[pos:pos + n] = sub[idx]
                        slot_rel[pos:pos + n] = rel[idx] * 64 + ridx[idx]
                        slot_obj[pos:pos + n] = sel[idx]
                    # pad slots: harmless gather target in the right half
                    slot_sub[pos + n:pos + nt] = 0 if h == 0 else HALF
                    pos += nt

            # per-slot arrays in [p, c] layout (slot j -> p=j%128, c=j//128)
            j = np.arange(n_all)
            objs[k, j % P, c_start + j // P] = slot_obj
            # idx arrays in 16-partition wrap, one wrap run per half segment
            jA = np.arange(nA)
            jB = np.arange(n_all - nA)
            subs16[k, jA % 16, s0 + jA // 16] = slot_sub[:nA]
            subs16[k, jB % 16, s0 + nA // 16 + jB // 16] = slot_sub[nA:] - HALF
            rels16[k, jA % 16, s0 + jA // 16] = slot_rel[:nA]
            rels16[k, jB % 16, s0 + nA // 16 + jB // 16] = slot_rel[nA:]

    subs16 = np.tile(subs16, (1, 8, 1))   # replicate to 128 partitions
    rels16 = np.tile(rels16, (1, 8, 1))
    return subs16, rels16, objs, tile_window, groups, ctot


DEBUG_OUTPUTS = False


def _build_graph(ctot, tile_window, groups, kpos):
    import concourse.bass as bass
    import concourse.bacc as bacc
    import concourse.mybir as mybir
    from concourse.tile import TileContext

    f32 = mybir.dt.float32
    bf16 = mybir.dt.bfloat16
    i16 = mybir.dt.int16
    AF = mybir.ActivationFunctionType
    Alu = mybir.AluOpType

    S = ctot * P // 16
    assert 2 <= kpos <= 126

    nc = bacc.Bacc(dynamic_dma_scratch_size=65536)
    hidT = nc.declare_dram_parameter("hidT", [P, ROWS_T], bf16, isOutput=False)
    relaT = nc.declare_dram_parameter("relaT", [P, 512], bf16, isOutput=False)
    qrelT = nc.declare_dram_parameter("qrelT", [P, 64], bf16, isOutput=False)
    ws_p = nc.declare_dram_parameter("ws_p", [D, D], bf16, isOutput=False)
    wr_p = nc.declare_dram_parameter("wr_p", [D, D], bf16, isOutput=False)
    wh_p = nc.declare_dram_parameter("wh_p", [D, D], bf16, isOutput=False)
    wqr_p = nc.declare_dram_parameter("wqr_p", [D, D], bf16, isOutput=False)
    b_p = nc.declare_dram_parameter("b_p", [1, D], bf16, isOutput=False)
    sub_i = nc.declare_dram_parameter("sub_i", [P, S], i16, isOutput=False)
    rel_i = nc.declare_dram_parameter("rel_i", [P, S], i16, isOutput=False)
    obj_f = nc.declare_dram_parameter("obj_f", [P, ctot], f32, isOutput=False)
    out_ext = nc.declare_dram_parameter("out", [OUT_ROWS, D], f32, isOutput=True)

    first_tile = {}
    last_tile = {}
    for c, w in enumerate(tile_window):
        if w not in first_tile:
            first_tile[w] = c
        last_tile[w] = c

    with TileContext(nc) as tc:
        with (
            tc.tile_pool(name="const", bufs=1) as cpool,
            tc.tile_pool(name="dram", bufs=1, space="DRAM") as dpool,
            tc.tile_pool(name="work", bufs=2) as wpool,
            tc.tile_pool(name="psum", bufs=4, space="PSUM") as ppool,
            tc.tile_pool(name="aggp", bufs=4, space="PSUM") as apool,
        ):
            # ---- constants ----
            chan_i = cpool.tile([P, 1], mybir.dt.int32)
            nc.gpsimd.iota(chan_i[:], pattern=[[0, 1]], base=0, channel_multiplier=1)
            chan_f = cpool.tile([P, 1], f32)
            nc.vector.tensor_copy(chan_f[:], chan_i[:])

            iota_i = cpool.tile([P, P], i16)
            nc.gpsimd.iota(iota_i[:], pattern=[[1, P]], base=0, channel_multiplier=0)
            iota_b = cpool.tile([P, P], bf16)
            nc.vector.tensor_copy(iota_b[:], iota_i[:])

            # E64[k, 64k'+b] = (k == k') (crel rel selector, sliced on free dim)
            e64 = cpool.tile([P, P * 64], bf16)
            e_scr = cpool.tile([P, 32 * 64], i16)
            e_scrb = cpool.tile([P, 32 * 64], bf16)
            for q in range(4):
                nc.gpsimd.iota(e_scr[:], pattern=[[1, 32], [0, 64]],
                               base=32 * q, channel_multiplier=0)
                nc.vector.tensor_copy(e_scrb[:], e_scr[:])
                nc.vector.tensor_scalar(
                    out=e64[:, q * 2048:(q + 1) * 2048], in0=e_scrb[:],
                    scalar1=chan_f[:], scalar2=None, op0=Alu.is_equal)

            # I64dup[q, j] = (q == j % 64)
            i64_i = cpool.tile([64, P], i16)
            nc.gpsimd.iota(i64_i[:], pattern=[[0, 2], [1, 64]], base=0,
                           channel_multiplier=0)
            i64_b = cpool.tile([64, P], bf16)
            nc.vector.tensor_copy(i64_b[:], i64_i[:])
            i64dup = cpool.tile([64, P], bf16)
            nc.vector.tensor_scalar(out=i64dup[:], in0=i64_b[:],
                                    scalar1=chan_f[0:64, :], scalar2=None,
                                    op0=Alu.is_equal)

            ones64 = cpool.tile([1, 64], bf16)
            nc.gpsimd.memset(ones64[:], 1.0)

            # ---- weights ----
            wcat_g = cpool.tile([P, 2 * D], bf16)
            nc.sync.dma_start(out=wcat_g[:, 0:D], in_=ws_p[:])
            nc.sync.dma_start(out=wcat_g[:, D:2 * D], in_=wh_p[:])
            wcat_r = cpool.tile([P, 2 * D], bf16)
            nc.sync.dma_start(out=wcat_r[:, 0:D], in_=wr_p[:])
            nc.sync.dma_start(out=wcat_r[:, D:2 * D], in_=wh_p[:])
            wqr_sb = cpool.tile([P, D], bf16)
            nc.sync.dma_start(out=wqr_sb[:], in_=wqr_p[:])
            bp_sb = cpool.tile([1, D], bf16)
            nc.sync.dma_start(out=bp_sb[:], in_=b_p[:])
            qrelT_sb = cpool.tile([P, 64], bf16)
            nc.sync.dma_start(out=qrelT_sb[:], in_=qrelT[:])
            relaT_sb = cpool.tile([P, 512], bf16)
            nc.sync.dma_start(out=relaT_sb[:], in_=relaT[:])

            # ---- edge index arrays ----
            sub_s = cpool.tile([P, S], i16)
            nc.sync.dma_start(out=sub_s[:], in_=sub_i[:])
            rel_s = cpool.tile([P, S], i16)
            nc.sync.dma_start(out=rel_s[:], in_=rel_i[:])
            obj_s = cpool.tile([P, ctot], f32)
            nc.sync.dma_start(out=obj_s[:], in_=obj_f[:])

            # ---- DRAM tables ----
            if DEBUG_OUTPUTS:
                hG_A = nc.declare_dram_parameter("dbg_hga", [HALF, 2 * D], bf16,
                                                 isOutput=True)
                hG_B = nc.declare_dram_parameter("dbg_hgb", [HALF, 2 * D], bf16,
                                                 isOutput=True)
                crel_d = nc.declare_dram_parameter("dbg_crel", [CREL_ROWS, 2 * D],
                                                   bf16, isOutput=True)
                dbg_l = nc.declare_dram_parameter("dbg_l", [P, 3 * ctot], f32,
                                                  isOutput=True)
            else:
                hG_A = dpool.tile([HALF, 2 * D], bf16)
                hG_B = dpool.tile([HALF, 2 * D], bf16)
                crel_d = dpool.tile([CREL_ROWS, 2 * D], bf16)

            # hrG in SBUF: [r_chunk, 256] x 4 chunks (512 rel rows padded)
            hrg_sb = cpool.tile([P, 4, 2 * D], bf16)
            for c in range(4):
                ps = ppool.tile([P, 2 * D], f32, tag="mm")
                nc.tensor.matmul(ps[:], lhsT=relaT_sb[:, c * P:(c + 1) * P],
                                 rhs=wcat_r[:], start=True, stop=True)
                nc.scalar.copy(hrg_sb[:, c, :], ps[:])

            # hqr' = qrel_sel @ Wqr' + b' -> [64, 256] (zero second half)
            hqr256 = cpool.tile([64, 2 * D], bf16)
            nc.gpsimd.memset(hqr256[:], 0.0)
            q_ps = ppool.tile([P, 2 * D], f32, tag="mm")
            nc.tensor.matmul(q_ps[0:64, 0:D], lhsT=qrelT_sb[:], rhs=wqr_sb[:],
                             start=True, stop=False)
            nc.tensor.matmul(q_ps[0:64, 0:D], lhsT=ones64[:], rhs=bp_sb[:],
                             start=False, stop=True)
            nc.scalar.copy(hqr256[:, 0:D], q_ps[0:64, 0:D])

            def batched_store(dst_tensor, row0, stage, nt):
                ap = bass.AP(dst_tensor, row0 * 2 * D,
                             [[2 * D, P], [P * 2 * D, nt], [1, 2 * D]])
                nc.sync.dma_start(out=ap, in_=stage[:, 0:nt, :])

            # ---- crel table build (PE); paired PSUM eviction on Act/DVE ----
            evict_n = 0

            def evict(dst_ap, src_ap):
                nonlocal evict_n
                if evict_n % 2 == 0:
                    nc.scalar.copy(dst_ap, src_ap)
                else:
                    nc.vector.tensor_copy(dst_ap, src_ap)
                evict_n += 1

            for t0 in range(0, CREL_T, 8):
                nt = min(8, CREL_T - t0)
                stage = wpool.tile([P, 8, 2 * D], bf16, tag="stage_c")
                for j in range(0, nt, 2):
                    np_ = min(2, nt - j)
                    ps = ppool.tile([P, np_ * 2 * D], f32, tag="mm")
                    for u in range(np_):
                        t = t0 + j + u
                        r0m = (2 * t) % P
                        chunk = (2 * t) // P
                        pcol = ps[:, u * 2 * D:(u + 1) * 2 * D]
                        nc.tensor.matmul(pcol, lhsT=e64[:, r0m * 64:r0m * 64 + P],
                                         rhs=hrg_sb[:, chunk, :],
                                         start=True, stop=False)
                        nc.tensor.matmul(pcol, lhsT=i64dup[:], rhs=hqr256[:],
                                         start=False, stop=True)
                    evict(stage[:, j:j + np_, :], ps[:])
                batched_store(crel_d[:].tensor, t0 * P, stage, nt)

            # ---- hG table build (PE), half A then half B ----
            for half_i, hG in ((0, hG_A), (1, hG_B)):
                for b0 in range(0, NT_H, BCH):
                    h_t = wpool.tile([P, BCH * P], bf16, tag="h_in")
                    col0 = (half_i * NT_H + b0) * P
                    nc.sync.dma_start(out=h_t[:], in_=hidT[:, col0:col0 + BCH * P])
                    stage = wpool.tile([P, BCH, 2 * D], bf16, tag="stage_g")
                    for j in range(0, BCH, 2):
                        ps = ppool.tile([P, 2 * 2 * D], f32, tag="mm")
                        for u in range(2):
                            nc.tensor.matmul(
                                ps[:, u * 2 * D:(u + 1) * 2 * D],
                                lhsT=h_t[:, (j + u) * P:(j + u + 1) * P],
                                rhs=wcat_g[:], start=True, stop=True)
                        evict(stage[:, j:j + 2, :], ps[:])
                    batched_store(hG[:].tensor, b0 * P, stage, BCH)

            # ---- per-edge-slot accumulators ----
            l1 = cpool.tile([P, ctot], f32)
            l2 = cpool.tile([P, ctot], f32)
            lg = cpool.tile([P, ctot], f32)
            alpha = cpool.tile([P, ctot], f32)

            def chunked_gather(dst_tile, src_ap, idxs_tile, idx_col0, t_off, n):
                done = 0
                while done < n:
                    cn = min(MAXI, n - done)
                    ct0 = t_off + done // P
                    nc.gpsimd.dma_gather(
                        out_ap=dst_tile[:, ct0:ct0 + cn // P, :],
                        in_ap=src_ap,
                        idxs_ap=idxs_tile[:, idx_col0 + done // 16:
                                          idx_col0 + (done + cn) // 16],
                        num_idxs=cn, num_idxs_reg=cn, elem_size=2 * D)
                    done += cn

            # ---- edge processing ----
            agg = {}
            for g_idx, (c_start, tA, tB) in enumerate(groups):
                g0w = g_idx * G
                T = tA + tB
                nA = tA * P
                nB = tB * P
                s0 = c_start * P // 16

                r_t = wpool.tile([P, T, 2 * D], bf16, tag="g_r", bufs=2)
                # s = g + r in place, one half-table segment at a time
                if tA:
                    gA = wpool.tile([P, tA, 2 * D], bf16, tag="g_g", bufs=3)
                    chunked_gather(gA, hG_A[:], sub_s, s0, 0, nA)
                    chunked_gather(r_t, crel_d[:], rel_s, s0, 0, nA)
                    nc.vector.tensor_tensor(out=r_t[:, 0:tA, :], in0=r_t[:, 0:tA, :],
                                            in1=gA[:, 0:tA, :], op=Alu.add)
                if tB:
                    gB = wpool.tile([P, tB, 2 * D], bf16, tag="g_g", bufs=3)
                    chunked_gather(gB, hG_B[:], sub_s, s0 + nA // 16, 0, nB)
                    chunked_gather(r_t, crel_d[:], rel_s, s0 + nA // 16, tA, nB)
                    nc.vector.tensor_tensor(out=r_t[:, tA:T, :], in0=r_t[:, tA:T, :],
                                            in1=gB[:, 0:tB, :], op=Alu.add)

                dump = wpool.tile([P, P], bf16, tag="dump")
                for c in range(T):
                    ct = c_start + c
                    nc.vector.tensor_scalar(
                        out=dump[:, 0:kpos], in0=r_t[:, c, 0:kpos],
                        scalar1=0.0, scalar2=0.0, op0=Alu.max, op1=Alu.add,
                        accum_out=l1[:, ct:ct + 1])
                    nc.vector.tensor_scalar(
                        out=dump[:, kpos:D], in0=r_t[:, c, kpos:D],
                        scalar1=0.0, scalar2=0.0, op0=Alu.max, op1=Alu.add,
                        accum_out=l2[:, ct:ct + 1])

                nc.vector.tensor_tensor(
                    out=lg[:, c_start:c_start + T], in0=l1[:, c_start:c_start + T],
                    in1=l2[:, c_start:c_start + T], op=Alu.subtract)
                nc.scalar.activation(alpha[:, c_start:c_start + T],
                                     lg[:, c_start:c_start + T], AF.Sigmoid)

                nw = len(set(tile_window[c_start:c_start + T]))
                ostage = wpool.tile([P, G, D], f32, tag="ostage")
                for c in range(T):
                    ct = c_start + c
                    oh = wpool.tile([P, P], bf16, tag="oh", bufs=4)
                    nc.vector.tensor_scalar(
                        out=oh[:], in0=iota_b[:],
                        scalar1=obj_s[:, ct:ct + 1], scalar2=alpha[:, ct:ct + 1],
                        op0=Alu.is_equal, op1=Alu.mult)
                    w = tile_window[ct]
                    if ct == first_tile[w]:
                        agg[w] = apool.tile([P, D], f32, tag="agg", name=f"agg_{w}")
                    nc.tensor.matmul(agg[w][:], lhsT=oh[:],
                                     rhs=r_t[:, c, D:2 * D],
                                     start=(ct == first_tile[w]),
                                     stop=(ct == last_tile[w]))
                    if ct == last_tile[w]:
                        nc.scalar.activation(ostage[:, w - g0w, :], agg[w][:],
                                             AF.Relu)
                        del agg[w]
                out_ap = bass.AP(out_ext[:].tensor, g0w * P * D,
                                 [[D, P], [P * D, nw], [1, D]])
                nc.sync.dma_start(out=out_ap, in_=ostage[:, 0:nw, :])

            if DEBUG_OUTPUTS:
                nc.sync.dma_start(out=dbg_l[:, 0:ctot], in_=l1[:])
                nc.sync.dma_start(out=dbg_l[:, ctot:2 * ctot], in_=l2[:])
                nc.sync.dma_start(out=dbg_l[:, 2 * ctot:3 * ctot], in_=alpha[:])

    nc.compile()
    return nc


def _prep_weights(Wa, Ws, Wr, Wqr_w, Wqr_b):
    import ml_dtypes
    wa = np.asarray(Wa, dtype=np.float64)
    pos = np.nonzero(wa >= 0)[0]
    neg = np.nonzero(wa < 0)[0]
    order = np.concatenate([pos, neg])
    kpos = len(pos)
    scale = np.abs(wa)[order]

    def prep(w):
        w = np.asarray(w, dtype=np.float64)[:, order] * scale[None, :]
        return np.ascontiguousarray(w.astype(ml_dtypes.bfloat16))

    ws_p = prep(Ws)
    wr_p = prep(Wr)
    wqr_p = prep(Wqr_w)
    b_p = (np.asarray(Wqr_b, dtype=np.float64)[order] * scale).reshape(1, D)
    b_p = np.ascontiguousarray(b_p.astype(ml_dtypes.bfloat16))
    return ws_p, wr_p, wqr_p, b_p, kpos


def prepare(q_rel, hidden, edges, rela_embed, Ws, Wr, Wqr_w, Wqr_b, Wa, Wh,
            n_node=None):
    """Build the Bass graph and the 8 per-core input maps."""
    import ml_dtypes

    q_rel = np.asarray(q_rel)
    hidden = np.asarray(hidden, dtype=np.float32)
    edges = np.asarray(edges)
    rela_embed = np.asarray(rela_embed, dtype=np.float32)

    subs16, rels16, objs, tile_window, groups, ctot = _host_shard(edges)
    ws_p, wr_p, wqr_p, b_p, kpos = _prep_weights(Wa, Ws, Wr, Wqr_w, Wqr_b)
    nc = _build_graph(ctot, tile_window, groups, kpos)

    bf = ml_dtypes.bfloat16
    hidT = np.zeros((D, ROWS_T), dtype=bf)
    hidT[:, :N] = hidden.T.astype(bf)
    relaT = np.zeros((D, 512), dtype=bf)
    relaT[:, :rela_embed.shape[0]] = rela_embed.T.astype(bf)
    qrelT = np.ascontiguousarray(
        rela_embed[np.asarray(q_rel, dtype=np.int64)].T.astype(bf))
    wh_b = np.ascontiguousarray(np.asarray(Wh, dtype=np.float32).astype(bf))

    in_maps = []
    for k in range(NCORES):
        in_maps.append({
            "hidT": hidT,
            "relaT": relaT,
            "qrelT": qrelT,
            "ws_p": ws_p,
            "wr_p": wr_p,
            "wh_p": wh_b,
            "wqr_p": wqr_p,
            "b_p": b_p,
            "sub_i": subs16[k],
            "rel_i": rels16[k],
            "obj_f": objs[k],
        })
    return nc, in_maps


def kernel(q_rel, hidden, edges, rela_embed, Ws, Wr, Wqr_w, Wqr_b, Wa, Wh, n_node):
    from concourse.bass_utils import run_bass_kernel_spmd

    nc, in_maps = prepare(q_rel, hidden, edges, rela_embed, Ws, Wr, Wqr_w,
                          Wqr_b, Wa, Wh, n_node)
    res = run_bass_kernel_spmd(nc, in_maps, list(range(NCORES)))
    out = np.concatenate([res.results[k]["out"][:NPC] for k in range(NCORES)],
                         axis=0)
    return out.astype(np.float32)


if __name__ == "__main__":
    import reference

    inputs = reference.setup_inputs()
    inputs = {k: np.asarray(v) for k, v in inputs.items()}
    got = kernel(**inputs)
    exp = np.asarray(reference.reference(**inputs))
    err = np.abs(got - exp).max() / (np.abs(exp).max() + 1e-9)
    print("rel err:", err)
